# revision 22
# baseline (speedup 1.0000x reference)
"""Bayesian GPLVM collapsed-ELBO kernel for Trainium2 (8 NeuronCores).

Sharding: data-parallel over n (2048 rows -> 256 per core). Each core
computes its partial psi2 = sum_n exp(log_psi2_n) over the 2080
upper-triangle (i,j) pairs, partial A = psi1^T y (64x256), and partial
row statistics (KL pieces, sum y^2). Host sums the 8 partials and does
the small m x m linear algebra to produce the scalar ELBO.

Device flow per core (n_loc = 256 as two 128-row chunks, fused):
  - 3 input DMAs: acqin (qmu/qls chunk-paired + alpha + consts + the
    psi1 z-side block), zl (psi2 z-side, 17 ij-chunks of 128 pairs),
    y (both chunks side by side).
  - prep runs fused over both chunks: one softplus, one Ln over
    [d1|d2|qsig], shared reduces; per-chunk tiles P hold the matmul
    rows [qmu*w1, w1, qmu*w2, w2, (1, g, h1)] which PE-transpose into
    nprep (80 x 256, q-major).
  - psi1 exponent = nprep[0:67,chunk]^T @ zs1x; A accumulates
    psi1^T y in PSUM and is DMA'd out mid-kernel, straight from PSUM.
  - psi2 exponent for each ij-chunk = zl-chunk^T @ nprep[32:66]; Exp on
    ACT in groups of 4 chunks; the n-sums run on DVE for 3 groups and
    as a halving-add tree on the (otherwise idle) Pool engine for 2.
  - All activations use only {Exp, Ln}: the activation-table universe
    passed to the table-load pass is filtered (indices preserved) so
    both are served by the one table that holds exp AND ln -> a single
    ACT_TABLE_LOAD that overlaps the input DMAs.
"""

import numpy as np

N, D, Q, M = 2048, 256, 16, 64
NCORES = 8
NLOC = N // NCORES          # 256

# acqin column layout
C_QM = 0        # [0:32)    qmu  (chunk0 | chunk1)
C_QLS = 32      # [32:64)   q_log_sigma
C_AL = 64      # [64:96)   alpha tiled twice
C_CONST = 96    # [96:100)  [2*logvar, 4*logvar, 0, 0]
ACQ_W = 104

_compiled = None


def _patch_act_tables():
    """Filter the activation-table universe handed to the table-load
    pass so `natural_log_exp_and_others` is the only table providing
    Exp/Ln. Table indices (act_func_set_id) are preserved, so the
    emitted loads still point at the real act_info.json entry; the
    whole kernel then needs a single ACT_TABLE_LOAD."""
    import concourse.bacc as bacc_mod
    import concourse.mybir as mybir
    from concourse.hw_specs import get_activation_tables

    def patched(arch):
        real = get_activation_tables(arch)
        target = None
        for name, funcs in real.items():
            if (mybir.ActivationFunctionType.Exp in funcs
                    and mybir.ActivationFunctionType.Ln in funcs):
                target = name
                break
        if target is None:
            return real
        strip = {mybir.ActivationFunctionType.Exp,
                 mybir.ActivationFunctionType.Ln}
        return {
            name: (set(funcs) if name == target else set(funcs) - strip)
            for name, funcs in real.items()
        }

    bacc_mod.get_activation_tables = patched


def _build_bass():
    import concourse.bacc as bacc
    import concourse.mybir as mybir
    from concourse import masks
    from concourse.tile import TileContext

    _patch_act_tables()

    f32 = mybir.dt.float32
    f32r = mybir.dt.float32r
    AF = mybir.ActivationFunctionType
    OP = mybir.AluOpType
    AX = mybir.AxisListType

    nc = bacc.Bacc("TRN2", target_bir_lowering=False, num_swdge_queues=2)

    acqin_d = nc.declare_dram_parameter("acqin", [128, ACQ_W], f32, isOutput=False)
    zl_d = nc.declare_dram_parameter("zl", [34, 17 * 128], f32r, isOutput=False)
    y_d = nc.declare_dram_parameter("ybig", [128, 2 * NLOC], f32r, isOutput=False)
    zs1_d = nc.declare_dram_parameter("zs1x", [33, M], f32r, isOutput=False)
    a_o = nc.declare_dram_parameter("out_A", [M, D], f32, isOutput=True)
    st_o = nc.declare_dram_parameter("out_stats", [128, 25], f32, isOutput=True)

    with TileContext(nc) as tc:
        with (
            tc.tile_pool(name="const", bufs=1) as cpool,
            tc.tile_pool(name="big", bufs=1) as bigpool,
            tc.tile_pool(name="scr", bufs=2) as spool,
            tc.tile_pool(name="psum", bufs=2, space="PSUM") as ppool,
            tc.tile_pool(name="psums", bufs=1, space="PSUM") as ppools,
            tc.tile_pool(name="psum1", bufs=1, space="PSUM") as ppool1,
        ):
            # inputs. The 34-partition zl transfer (the per-partition-
            # bytes hog) is split: first 9 ij-chunks ride the sync (SP)
            # ring ahead of y, the rest go on the Scalar HW-DGE ring,
            # issued after the prep activations so the issue cost sits in
            # an ACT idle window.
            acqin = cpool.tile([128, ACQ_W], f32)
            nc.sync.dma_start(out=acqin[:, :], in_=acqin_d[:, :])
            zs1_sb = cpool.tile([33, M], f32r)
            nc.sync.dma_start(out=zs1_sb[:, :], in_=zs1_d[:, :])
            zl_sb = bigpool.tile([98, 17 * 128], f32r)
            zhalf = 9 * 128
            nc.sync.dma_start(out=zl_sb[64:98, :zhalf], in_=zl_d[:, :zhalf])
            ybig = bigpool.tile([128, 2 * NLOC], f32r)
            nc.sync.dma_start(out=ybig[:, :], in_=y_d[:, :])
            nc.sync.dma_start(out=zl_sb[64:98, zhalf:], in_=zl_d[:, zhalf:])

            ident = cpool.tile([128, 128], f32)
            masks.make_identity(nc, ident[:])

            qm = acqin[:, C_QM:C_QM + 32]
            qls = acqin[:, C_QLS:C_QLS + 32]
            al2 = acqin[:, C_AL:C_AL + 32]
            c2lv = acqin[:, C_CONST:C_CONST + 1]
            zs1x = zs1_sb[:, :]

            stats = bigpool.tile([128, 25], f32)
            nprep = bigpool.tile([98, 2 * 128], f32r)
            P0 = bigpool.tile([128, 98], f32)
            P1 = bigpool.tile([128, 98], f32)
            S = bigpool.tile([128, 96], f32)      # [d1(32) | d2(32) | qsig(32)]
            L = bigpool.tile([128, 96], f32)      # ln of S
            R = bigpool.tile([128, 64], f32)      # 1/d1, 1/d2
            M1 = bigpool.tile([128, 32], f32)
            scr2 = bigpool.tile([128, 64], f32)
            cols = bigpool.tile([128, 8], f32)

            qsig = S[:, 64:96]
            d1 = S[:, 0:32]
            d2 = S[:, 32:64]

            # q_sigma = softplus(qls) = ln(1 + exp(qls)), both chunks
            nc.scalar.activation(M1[:, :], qls, AF.Exp)
            nc.scalar.activation(qsig, M1[:, :], AF.Ln, bias=1.0)
            # d1 = alpha*qsig + 1 ; d2 = 2*d1 - 1
            nc.vector.tensor_mul(M1[:, :], qsig, al2)
            nc.vector.tensor_scalar_add(d1, M1[:, :], 1.0)
            nc.vector.tensor_scalar(out=d2, in0=d1, scalar1=2.0, scalar2=-1.0,
                                    op0=OP.mult, op1=OP.add)
            nc.vector.reciprocal(R[:, :], S[:, 0:64])
            # one Ln serves sum2 (ln d1), 2*s3 (ln d2) and the KL ln qsig
            nc.scalar.activation(L[:, :], S[:, :], AF.Ln)
            # SR cols: [sum2_c0, sum2_c1, s3x2_c0, s3x2_c1, lnsig_c0, lnsig_c1]
            nc.vector.tensor_reduce(
                stats[:, 17:23], L[:, :].rearrange("p (a b) -> p a b", b=16),
                axis=AX.X, op=OP.add)

            for c, P in enumerate((P0, P1)):
                sl = slice(16 * c, 16 * c + 16)
                qmc = acqin[:, C_QM + 16 * c:C_QM + 16 * c + 16]
                alc = acqin[:, C_AL:C_AL + 16]
                # rows (post-transpose): 0:16 qmu*w1, 16:32 w1, 32 h1,
                # 33:64 zero, 64:80 qmu*w2, 80:96 w2, 96 one, 97 g
                nc.vector.tensor_mul(P[:, 16:32], R[:, sl], alc)
                nc.vector.tensor_mul(P[:, 80:96], R[:, 32:][:, sl], alc)
                nc.vector.tensor_mul(P[:, 0:16], P[:, 16:32], qmc)
                nc.vector.tensor_mul(P[:, 64:80], P[:, 80:96], qmc)
                nc.vector.tensor_mul(scr2[:, 0:16], P[:, 0:16], qmc)
                nc.vector.tensor_mul(scr2[:, 16:32], P[:, 64:80], qmc)
                # rt1 = sum_q qmu^2 w1 ; a = sum_q qmu^2 w2
                nc.vector.tensor_reduce(
                    cols[:, 2 * c:2 * c + 2],
                    scr2[:, 0:32].rearrange("p (a b) -> p a b", b=16),
                    axis=AX.X, op=OP.add)
                rt1c = cols[:, 2 * c:2 * c + 1]
                ac = cols[:, 2 * c + 1:2 * c + 2]
                sum2c = stats[:, 17 + c:18 + c]
                s3x2c = stats[:, 19 + c:20 + c]
                nc.vector.memset(P[:, 33:64], 0.0)
                nc.vector.memset(P[:, 96:97], 1.0)
                # h1 = 2*logvar - 0.5*(rt1 + sum2)
                nc.vector.tensor_add(cols[:, 4 + c:5 + c], rt1c, sum2c)
                nc.vector.tensor_scalar(
                    out=P[:, 32:33], in0=cols[:, 4 + c:5 + c], scalar1=-0.5,
                    scalar2=c2lv, op0=OP.mult, op1=OP.add)
                # g = -a - 0.5*(2*s3); the 4*logvar constant rides in
                # the zl "ones" row instead
                nc.vector.tensor_scalar(
                    out=cols[:, 6 + c:7 + c], in0=s3x2c, scalar1=0.5,
                    scalar2=ac, op0=OP.mult, op1=OP.add)
                nc.vector.tensor_scalar(
                    out=P[:, 97:98], in0=cols[:, 6 + c:7 + c], scalar1=-1.0,
                    scalar2=0.0, op0=OP.mult, op1=OP.add)

                ptp = ppools.tile([98, 128], f32, tag="ptp")
                nc.tensor.transpose(ptp[:, :], P[:, :], ident[:, :])
                nc.vector.tensor_copy(nprep[:, 128 * c:128 * (c + 1)], ptp[:, :])

            # psi1 exponent for both chunks into one PSUM tile, then exp
            e1 = ppools.tile([128, 2 * M], f32, tag="e1")
            for c in range(2):
                nc.tensor.matmul(e1[:, M * c:M * (c + 1)],
                                 lhsT=nprep[0:33, 128 * c:128 * (c + 1)],
                                 rhs=zs1x,
                                 start=True, stop=True)
            psi1c = bigpool.tile([128, 2 * M], f32r)
            nc.scalar.activation(psi1c[:, :], e1[:, :], AF.Exp)

            # KL statistics (tr(y y^T) is an input-only reduction and is
            # done on the host)
            nc.vector.tensor_mul(scr2[:, 0:32], qm, qm)
            nc.vector.tensor_mul(scr2[:, 32:64], qsig, qsig)
            nc.vector.tensor_reduce(
                stats[:, 23:25], scr2[:, :].rearrange("p (a b) -> p a b", b=32),
                axis=AX.X, op=OP.add)

            # psi2: 17 ij-chunks in groups (first group small so the ACT
            # exp stream starts as early as possible); n-sums on DVE
            GROUPS = (2, 4, 4, 4, 2, 1)
            ch0 = 0
            for nch in GROUPS:
                w = nch * NLOC
                p2 = ppool.tile([128, 4 * NLOC], f32, tag="p2")
                for j in range(nch):
                    ch = ch0 + j
                    nc.tensor.matmul(
                        p2[:, j * NLOC:(j + 1) * NLOC],
                        lhsT=zl_sb[64:98, ch * 128:(ch + 1) * 128],
                        rhs=nprep[64:98, :],
                        start=True, stop=True)
                scr = spool.tile([128, 4 * NLOC], f32, tag="p2scr")
                nc.scalar.activation(scr[:, :w], p2[:, :w], AF.Exp)
                nc.vector.tensor_reduce(
                    stats[:, ch0:ch0 + nch],
                    scr[:, :w].rearrange("p (a b) -> p a b", b=NLOC),
                    axis=AX.X, op=OP.add)
                ch0 += nch

            # A = psi1^T y: PE work ordered after the psi2 matmuls so the
            # exp stream starts earlier; the DMA still overlaps the tail
            apsum = ppool1.tile([M, D], f32)
            for c in range(2):
                nc.tensor.matmul(apsum[:, :],
                                 lhsT=psi1c[:, M * c:M * (c + 1)],
                                 rhs=ybig[:, NLOC * c:NLOC * (c + 1)],
                                 start=(c == 0), stop=(c == 1))
            a_sb = bigpool.tile([M, D], f32)
            nc.vector.tensor_copy(a_sb[:, :], apsum[:, :])
            nc.sync.dma_start(out=a_o[:, :], in_=a_sb[:, :])
            nc.sync.dma_start(out=st_o[:, :], in_=stats[:, :])

    nc.compile()
    return nc


def _get_compiled():
    global _compiled
    if _compiled is None:
        _compiled = _build_bass()
    return _compiled


def _np_softplus(x):
    return np.logaddexp(x, 0.0)


def kernel(y, q_mu, q_log_sigma, z, noise_raw, alpha, variance, _trace=False):
    from concourse.bass_utils import run_bass_kernel_spmd

    nc = _get_compiled()

    f8 = np.float64
    z64 = z.astype(f8)
    al = alpha.astype(f8)
    var = f8(variance[0])
    logvar = np.log(var)

    # z-side stationary blocks (host-built, replicated to all cores).
    # psi2 is symmetric in (i, j): ship only the 2080 upper-tri pairs.
    iu, ju = np.triu_indices(M)                             # (2080,)
    npairs = iu.shape[0]
    Su = z64[iu] + z64[ju]                                  # (2080, q)
    sqz = (z64[:, None, :] - z64[None, :, :]) ** 2          # (m, m, q)
    s1 = 0.25 * (sqz @ al)                                  # (m, m)
    zl = np.zeros((34, 17 * 128), np.float32)
    zl[0:16, :npairs] = Su.T
    zl[16:32, :npairs] = (-0.25 * Su * Su).T
    zl[32, :npairs] = -s1[iu, ju] + 4.0 * logvar
    zl[33, :npairs] = 1.0

    zt = z64.T                                              # (q, m)
    zs1x = np.zeros((33, M), np.float32)
    zs1x[0:16] = zt
    zs1x[16:32] = -0.5 * zt * zt
    zs1x[32] = 1.0

    qmu32 = q_mu.astype(np.float32)
    qls32 = q_log_sigma.astype(np.float32)
    y32 = y.astype(np.float32)

    in_maps = []
    for i in range(NCORES):
        r = i * NLOC
        acqin = np.zeros((128, ACQ_W), np.float32)
        acqin[:, C_QM:C_QM + 16] = qmu32[r:r + 128]
        acqin[:, C_QM + 16:C_QM + 32] = qmu32[r + 128:r + 256]
        acqin[:, C_QLS:C_QLS + 16] = qls32[r:r + 128]
        acqin[:, C_QLS + 16:C_QLS + 32] = qls32[r + 128:r + 256]
        acqin[:, C_AL:C_AL + 16] = alpha.reshape(1, Q)
        acqin[:, C_AL + 16:C_AL + 32] = alpha.reshape(1, Q)
        acqin[:, C_CONST] = 2.0 * logvar
        ybig = np.empty((128, 2 * NLOC), np.float32)
        ybig[:, 0:NLOC] = y32[r:r + 128]
        ybig[:, NLOC:2 * NLOC] = y32[r + 128:r + 256]
        in_maps.append({"acqin": acqin, "zl": zl, "ybig": ybig, "zs1x": zs1x})

    br = run_bass_kernel_spmd(nc, in_maps, list(range(NCORES)), trace=_trace)
    res = br.results

    stats = np.zeros((128, 25), f8)
    A = np.zeros((M, D), f8)
    for r in res:
        stats += r["out_stats"].astype(f8)
        A += r["out_A"].astype(f8)

    flat = stats[:, 0:17].T.reshape(17 * 128)
    psi2 = np.empty((M, M), f8)
    psi2[iu, ju] = flat[:npairs]
    psi2[ju, iu] = flat[:npairs]
    col = stats.sum(axis=0)
    lnsig = col[21] + col[22]
    musq = col[23]
    ssq = col[24]
    tr_yy = float(np.sum(y.astype(f8) ** 2))

    kl_sum = -lnsig + 0.5 * (ssq + musq) - 0.5 * N * Q
    kl_term = kl_sum / (N * D)

    # small m x m algebra on host
    k_mm = var * np.exp(-0.5 * (sqz @ al))                  # (m, m)
    noise_var = _np_softplus(f8(noise_raw[0]))
    beta = 1.0 / noise_var
    psi0 = N * var

    cov1 = beta * psi2 + k_mm
    B = np.linalg.solve(cov1, A)
    tr_yWy = beta * tr_yy - np.sum(A * B)

    F = 0.5 * N * np.log(beta)
    F += 0.5 * np.linalg.slogdet(k_mm)[1]
    F -= 0.5 * N * np.log(np.pi)
    F -= 0.5 * np.linalg.slogdet(cov1)[1]
    F -= 0.5 * beta * psi0
    F += 0.5 * np.trace(np.linalg.solve(k_mm, psi2))
    F = (F * D - 0.5 * tr_yWy) / (N * D)

    out = F - kl_term
    result = np.asarray(out, dtype=np.float32)
    if _trace:
        return result, br
    return result


# revision 24
# speedup vs baseline: 1.0276x; 1.0276x over previous
"""Bayesian GPLVM collapsed-ELBO kernel for Trainium2 (8 NeuronCores).

Sharding: data-parallel over n (2048 rows -> 256 per core). Each core
computes its partial psi2 = sum_n exp(log_psi2_n) over the 2080
upper-triangle (i,j) pairs, partial A = psi1^T y (64x256), and partial
row statistics (KL pieces, sum y^2). Host sums the 8 partials and does
the small m x m linear algebra to produce the scalar ELBO.

Device flow per core (n_loc = 256 as two 128-row chunks, fused):
  - 3 input DMAs: acqin (qmu/qls chunk-paired + alpha + consts + the
    psi1 z-side block), zl (psi2 z-side, 17 ij-chunks of 128 pairs),
    y (both chunks side by side).
  - prep runs fused over both chunks: one softplus, one Ln over
    [d1|d2|qsig], shared reduces; per-chunk tiles P hold the matmul
    rows [qmu*w1, w1, qmu*w2, w2, (1, g, h1)] which PE-transpose into
    nprep (80 x 256, q-major).
  - psi1 exponent = nprep[0:67,chunk]^T @ zs1x; A accumulates
    psi1^T y in PSUM and is DMA'd out mid-kernel, straight from PSUM.
  - psi2 exponent for each ij-chunk = zl-chunk^T @ nprep[32:66]; Exp on
    ACT in groups of 4 chunks; the n-sums run on DVE for 3 groups and
    as a halving-add tree on the (otherwise idle) Pool engine for 2.
  - All activations use only {Exp, Ln}: the activation-table universe
    passed to the table-load pass is filtered (indices preserved) so
    both are served by the one table that holds exp AND ln -> a single
    ACT_TABLE_LOAD that overlaps the input DMAs.
"""

import numpy as np

N, D, Q, M = 2048, 256, 16, 64
NCORES = 8
NLOC = N // NCORES          # 256

# acqin column layout
C_QM = 0        # [0:32)    qmu  (chunk0 | chunk1)
C_QLS = 32      # [32:64)   q_log_sigma
C_AL = 64      # [64:96)   alpha tiled twice
C_CONST = 96    # [96:100)  [2*logvar, 4*logvar, 0, 0]
ACQ_W = 104

_compiled = None


def _patch_act_tables():
    """Filter the activation-table universe handed to the table-load
    pass so `natural_log_exp_and_others` is the only table providing
    Exp/Ln. Table indices (act_func_set_id) are preserved, so the
    emitted loads still point at the real act_info.json entry; the
    whole kernel then needs a single ACT_TABLE_LOAD."""
    import concourse.bacc as bacc_mod
    import concourse.mybir as mybir
    from concourse.hw_specs import get_activation_tables

    def patched(arch):
        real = get_activation_tables(arch)
        target = None
        for name, funcs in real.items():
            if (mybir.ActivationFunctionType.Exp in funcs
                    and mybir.ActivationFunctionType.Ln in funcs):
                target = name
                break
        if target is None:
            return real
        strip = {mybir.ActivationFunctionType.Exp,
                 mybir.ActivationFunctionType.Ln}
        return {
            name: (set(funcs) if name == target else set(funcs) - strip)
            for name, funcs in real.items()
        }

    bacc_mod.get_activation_tables = patched


def _build_bass():
    import concourse.bacc as bacc
    import concourse.mybir as mybir
    from concourse import masks
    from concourse.tile import TileContext

    _patch_act_tables()

    f32 = mybir.dt.float32
    f32r = mybir.dt.float32r
    AF = mybir.ActivationFunctionType
    OP = mybir.AluOpType
    AX = mybir.AxisListType

    nc = bacc.Bacc("TRN2", target_bir_lowering=False, num_swdge_queues=2)

    acqin_d = nc.declare_dram_parameter("acqin", [128, ACQ_W], f32, isOutput=False)
    zl_d = nc.declare_dram_parameter("zl", [34, 17 * 128], f32r, isOutput=False)
    y_d = nc.declare_dram_parameter("ybig", [128, 2 * NLOC], f32r, isOutput=False)
    zs1_d = nc.declare_dram_parameter("zs1x", [33, M], f32r, isOutput=False)
    a_o = nc.declare_dram_parameter("out_A", [M, D], f32, isOutput=True)
    st_o = nc.declare_dram_parameter("out_stats", [128, 25], f32, isOutput=True)

    with TileContext(nc) as tc:
        with (
            tc.tile_pool(name="const", bufs=1) as cpool,
            tc.tile_pool(name="big", bufs=1) as bigpool,
            tc.tile_pool(name="scr", bufs=2) as spool,
            tc.tile_pool(name="psum", bufs=3, space="PSUM") as ppool,
            tc.tile_pool(name="psums", bufs=1, space="PSUM") as ppools,
        ):
            # inputs. The 34-partition zl transfer (the per-partition-
            # bytes hog) is split: first 9 ij-chunks ride the sync (SP)
            # ring ahead of y, the rest go on the Scalar HW-DGE ring,
            # issued after the prep activations so the issue cost sits in
            # an ACT idle window.
            acqin = cpool.tile([128, ACQ_W], f32)
            nc.sync.dma_start(out=acqin[:, :], in_=acqin_d[:, :])
            zs1_sb = cpool.tile([33, M], f32r)
            nc.sync.dma_start(out=zs1_sb[:, :], in_=zs1_d[:, :])
            zl_sb = bigpool.tile([98, 17 * 128], f32r)
            zhalf = 9 * 128
            nc.sync.dma_start(out=zl_sb[64:98, :zhalf], in_=zl_d[:, :zhalf])
            ybig = bigpool.tile([128, 2 * NLOC], f32r)
            nc.sync.dma_start(out=ybig[:, :], in_=y_d[:, :])
            nc.sync.dma_start(out=zl_sb[64:98, zhalf:], in_=zl_d[:, zhalf:])

            ident = cpool.tile([128, 128], f32)
            masks.make_identity(nc, ident[:])

            qm = acqin[:, C_QM:C_QM + 32]
            qls = acqin[:, C_QLS:C_QLS + 32]
            al2 = acqin[:, C_AL:C_AL + 32]
            c2lv = acqin[:, C_CONST:C_CONST + 1]
            zs1x = zs1_sb[:, :]

            stats = bigpool.tile([128, 25], f32)
            nprep = bigpool.tile([98, 2 * 128], f32r)
            P0 = bigpool.tile([128, 98], f32)
            P1 = bigpool.tile([128, 98], f32)
            S = bigpool.tile([128, 96], f32)      # [d1(32) | d2(32) | qsig(32)]
            L = bigpool.tile([128, 96], f32)      # ln of S
            R = bigpool.tile([128, 64], f32)      # 1/d1, 1/d2
            M1 = bigpool.tile([128, 32], f32)
            scr2 = bigpool.tile([128, 64], f32)
            cols = bigpool.tile([128, 8], f32)

            qsig = S[:, 64:96]
            d1 = S[:, 0:32]
            d2 = S[:, 32:64]

            # q_sigma = softplus(qls) = ln(1 + exp(qls)), both chunks
            nc.scalar.activation(M1[:, :], qls, AF.Exp)
            nc.scalar.activation(qsig, M1[:, :], AF.Ln, bias=1.0)
            # d1 = alpha*qsig + 1 ; d2 = 2*d1 - 1
            nc.vector.tensor_mul(M1[:, :], qsig, al2)
            nc.vector.tensor_scalar_add(d1, M1[:, :], 1.0)
            nc.vector.tensor_scalar(out=d2, in0=d1, scalar1=2.0, scalar2=-1.0,
                                    op0=OP.mult, op1=OP.add)
            nc.vector.reciprocal(R[:, :], S[:, 0:64])
            # one Ln serves sum2 (ln d1), 2*s3 (ln d2) and the KL ln qsig
            nc.scalar.activation(L[:, :], S[:, :], AF.Ln)
            # SR cols: [sum2_c0, sum2_c1, s3x2_c0, s3x2_c1, lnsig_c0, lnsig_c1]
            nc.vector.tensor_reduce(
                stats[:, 17:23], L[:, :].rearrange("p (a b) -> p a b", b=16),
                axis=AX.X, op=OP.add)

            for c, P in enumerate((P0, P1)):
                sl = slice(16 * c, 16 * c + 16)
                qmc = acqin[:, C_QM + 16 * c:C_QM + 16 * c + 16]
                alc = acqin[:, C_AL:C_AL + 16]
                # rows (post-transpose): 0:16 qmu*w1, 16:32 w1, 32 h1,
                # 33:64 zero, 64:80 qmu*w2, 80:96 w2, 96 one, 97 g
                nc.vector.tensor_mul(P[:, 16:32], R[:, sl], alc)
                nc.vector.tensor_mul(P[:, 80:96], R[:, 32:][:, sl], alc)
                nc.vector.tensor_mul(P[:, 0:16], P[:, 16:32], qmc)
                nc.vector.tensor_mul(P[:, 64:80], P[:, 80:96], qmc)
                nc.vector.tensor_mul(scr2[:, 0:16], P[:, 0:16], qmc)
                nc.vector.tensor_mul(scr2[:, 16:32], P[:, 64:80], qmc)
                # rt1 = sum_q qmu^2 w1 ; a = sum_q qmu^2 w2
                nc.vector.tensor_reduce(
                    cols[:, 2 * c:2 * c + 2],
                    scr2[:, 0:32].rearrange("p (a b) -> p a b", b=16),
                    axis=AX.X, op=OP.add)
                rt1c = cols[:, 2 * c:2 * c + 1]
                ac = cols[:, 2 * c + 1:2 * c + 2]
                sum2c = stats[:, 17 + c:18 + c]
                s3x2c = stats[:, 19 + c:20 + c]
                nc.vector.memset(P[:, 33:64], 0.0)
                nc.vector.memset(P[:, 96:97], 1.0)
                # h1 = 2*logvar - 0.5*(rt1 + sum2)
                nc.vector.tensor_add(cols[:, 4 + c:5 + c], rt1c, sum2c)
                nc.vector.tensor_scalar(
                    out=P[:, 32:33], in0=cols[:, 4 + c:5 + c], scalar1=-0.5,
                    scalar2=c2lv, op0=OP.mult, op1=OP.add)
                # g = -a - 0.5*(2*s3); the 4*logvar constant rides in
                # the zl "ones" row instead
                nc.vector.tensor_scalar(
                    out=cols[:, 6 + c:7 + c], in0=s3x2c, scalar1=0.5,
                    scalar2=ac, op0=OP.mult, op1=OP.add)
                nc.vector.tensor_scalar(
                    out=P[:, 97:98], in0=cols[:, 6 + c:7 + c], scalar1=-1.0,
                    scalar2=0.0, op0=OP.mult, op1=OP.add)

                ptp = ppools.tile([98, 128], f32, tag="ptp")
                nc.tensor.transpose(ptp[:, :], P[:, :], ident[:, :])
                nc.vector.tensor_copy(nprep[:, 128 * c:128 * (c + 1)], ptp[:, :])

            # psi1 exponent for both chunks into one PSUM tile, then exp
            # (borrows a rotation slot of the psi2 PSUM pool)
            e1 = ppool.tile([128, 4 * NLOC], f32, tag="p2")
            for c in range(2):
                nc.tensor.matmul(e1[:, M * c:M * (c + 1)],
                                 lhsT=nprep[0:33, 128 * c:128 * (c + 1)],
                                 rhs=zs1x,
                                 start=True, stop=True)
            psi1c = bigpool.tile([128, 2 * M], f32r)
            nc.scalar.activation(psi1c[:, :], e1[:, 0:2 * M], AF.Exp)

            # KL statistics (tr(y y^T) is an input-only reduction and is
            # done on the host)
            nc.vector.tensor_mul(scr2[:, 0:32], qm, qm)
            nc.vector.tensor_mul(scr2[:, 32:64], qsig, qsig)
            nc.vector.tensor_reduce(
                stats[:, 23:25], scr2[:, :].rearrange("p (a b) -> p a b", b=32),
                axis=AX.X, op=OP.add)

            # psi2: 17 ij-chunks in groups (first group small so the ACT
            # exp stream starts as early as possible); n-sums on DVE
            GROUPS = (2, 4, 4, 4, 2, 1)
            ch0 = 0
            for nch in GROUPS:
                w = nch * NLOC
                p2 = ppool.tile([128, 4 * NLOC], f32, tag="p2")
                for j in range(nch):
                    ch = ch0 + j
                    nc.tensor.matmul(
                        p2[:, j * NLOC:(j + 1) * NLOC],
                        lhsT=zl_sb[64:98, ch * 128:(ch + 1) * 128],
                        rhs=nprep[64:98, :],
                        start=True, stop=True)
                scr = spool.tile([128, 4 * NLOC], f32, tag="p2scr")
                nc.scalar.activation(scr[:, :w], p2[:, :w], AF.Exp)
                nc.vector.tensor_reduce(
                    stats[:, ch0:ch0 + nch],
                    scr[:, :w].rearrange("p (a b) -> p a b", b=NLOC),
                    axis=AX.X, op=OP.add)
                ch0 += nch

            # A = psi1^T y: PE work ordered after the psi2 matmuls so the
            # exp stream starts earlier; the DMA still overlaps the tail
            apsum = ppool.tile([128, 4 * NLOC], f32, tag="p2")
            for c in range(2):
                nc.tensor.matmul(apsum[0:M, 0:D],
                                 lhsT=psi1c[:, M * c:M * (c + 1)],
                                 rhs=ybig[:, NLOC * c:NLOC * (c + 1)],
                                 start=(c == 0), stop=(c == 1))
            a_sb = bigpool.tile([M, D], f32)
            nc.vector.tensor_copy(a_sb[:, :], apsum[0:M, 0:D])
            nc.sync.dma_start(out=a_o[:, :], in_=a_sb[:, :])
            nc.sync.dma_start(out=st_o[:, :], in_=stats[:, :])

    nc.compile()
    return nc


def _get_compiled():
    global _compiled
    if _compiled is None:
        _compiled = _build_bass()
    return _compiled


def _np_softplus(x):
    return np.logaddexp(x, 0.0)


def kernel(y, q_mu, q_log_sigma, z, noise_raw, alpha, variance, _trace=False):
    from concourse.bass_utils import run_bass_kernel_spmd

    nc = _get_compiled()

    f8 = np.float64
    z64 = z.astype(f8)
    al = alpha.astype(f8)
    var = f8(variance[0])
    logvar = np.log(var)

    # z-side stationary blocks (host-built, replicated to all cores).
    # psi2 is symmetric in (i, j): ship only the 2080 upper-tri pairs.
    iu, ju = np.triu_indices(M)                             # (2080,)
    npairs = iu.shape[0]
    Su = z64[iu] + z64[ju]                                  # (2080, q)
    sqz = (z64[:, None, :] - z64[None, :, :]) ** 2          # (m, m, q)
    s1 = 0.25 * (sqz @ al)                                  # (m, m)
    zl = np.zeros((34, 17 * 128), np.float32)
    zl[0:16, :npairs] = Su.T
    zl[16:32, :npairs] = (-0.25 * Su * Su).T
    zl[32, :npairs] = -s1[iu, ju] + 4.0 * logvar
    zl[33, :npairs] = 1.0

    zt = z64.T                                              # (q, m)
    zs1x = np.zeros((33, M), np.float32)
    zs1x[0:16] = zt
    zs1x[16:32] = -0.5 * zt * zt
    zs1x[32] = 1.0

    qmu32 = q_mu.astype(np.float32)
    qls32 = q_log_sigma.astype(np.float32)
    y32 = y.astype(np.float32)

    in_maps = []
    for i in range(NCORES):
        r = i * NLOC
        acqin = np.zeros((128, ACQ_W), np.float32)
        acqin[:, C_QM:C_QM + 16] = qmu32[r:r + 128]
        acqin[:, C_QM + 16:C_QM + 32] = qmu32[r + 128:r + 256]
        acqin[:, C_QLS:C_QLS + 16] = qls32[r:r + 128]
        acqin[:, C_QLS + 16:C_QLS + 32] = qls32[r + 128:r + 256]
        acqin[:, C_AL:C_AL + 16] = alpha.reshape(1, Q)
        acqin[:, C_AL + 16:C_AL + 32] = alpha.reshape(1, Q)
        acqin[:, C_CONST] = 2.0 * logvar
        ybig = np.empty((128, 2 * NLOC), np.float32)
        ybig[:, 0:NLOC] = y32[r:r + 128]
        ybig[:, NLOC:2 * NLOC] = y32[r + 128:r + 256]
        in_maps.append({"acqin": acqin, "zl": zl, "ybig": ybig, "zs1x": zs1x})

    br = run_bass_kernel_spmd(nc, in_maps, list(range(NCORES)), trace=_trace)
    res = br.results

    stats = np.zeros((128, 25), f8)
    A = np.zeros((M, D), f8)
    for r in res:
        stats += r["out_stats"].astype(f8)
        A += r["out_A"].astype(f8)

    flat = stats[:, 0:17].T.reshape(17 * 128)
    psi2 = np.empty((M, M), f8)
    psi2[iu, ju] = flat[:npairs]
    psi2[ju, iu] = flat[:npairs]
    col = stats.sum(axis=0)
    lnsig = col[21] + col[22]
    musq = col[23]
    ssq = col[24]
    tr_yy = float(np.sum(y.astype(f8) ** 2))

    kl_sum = -lnsig + 0.5 * (ssq + musq) - 0.5 * N * Q
    kl_term = kl_sum / (N * D)

    # small m x m algebra on host
    k_mm = var * np.exp(-0.5 * (sqz @ al))                  # (m, m)
    noise_var = _np_softplus(f8(noise_raw[0]))
    beta = 1.0 / noise_var
    psi0 = N * var

    cov1 = beta * psi2 + k_mm
    B = np.linalg.solve(cov1, A)
    tr_yWy = beta * tr_yy - np.sum(A * B)

    F = 0.5 * N * np.log(beta)
    F += 0.5 * np.linalg.slogdet(k_mm)[1]
    F -= 0.5 * N * np.log(np.pi)
    F -= 0.5 * np.linalg.slogdet(cov1)[1]
    F -= 0.5 * beta * psi0
    F += 0.5 * np.trace(np.linalg.solve(k_mm, psi2))
    F = (F * D - 0.5 * tr_yWy) / (N * D)

    out = F - kl_term
    result = np.asarray(out, dtype=np.float32)
    if _trace:
        return result, br
    return result


# revision 25
# speedup vs baseline: 1.0774x; 1.0484x over previous
"""Bayesian GPLVM collapsed-ELBO kernel for Trainium2 (8 NeuronCores).

Sharding: data-parallel over n (2048 rows -> 256 per core). Each core
computes its partial psi2 = sum_n exp(log_psi2_n) over the 2080
upper-triangle (i,j) pairs, partial A = psi1^T y (64x256), and partial
row statistics (KL pieces, sum y^2). Host sums the 8 partials and does
the small m x m linear algebra to produce the scalar ELBO.

Device flow per core (n_loc = 256 as two 128-row chunks, fused):
  - 3 input DMAs: acqin (qmu/qls chunk-paired + alpha + consts + the
    psi1 z-side block), zl (psi2 z-side, 17 ij-chunks of 128 pairs),
    y (both chunks side by side).
  - prep runs fused over both chunks: one softplus, one Ln over
    [d1|d2|qsig], shared reduces; per-chunk tiles P hold the matmul
    rows [qmu*w1, w1, qmu*w2, w2, (1, g, h1)] which PE-transpose into
    nprep (80 x 256, q-major).
  - psi1 exponent = nprep[0:67,chunk]^T @ zs1x; A accumulates
    psi1^T y in PSUM and is DMA'd out mid-kernel, straight from PSUM.
  - psi2 exponent for each ij-chunk = zl-chunk^T @ nprep[32:66]; Exp on
    ACT in groups of 4 chunks; the n-sums run on DVE for 3 groups and
    as a halving-add tree on the (otherwise idle) Pool engine for 2.
  - All activations use only {Exp, Ln}: the activation-table universe
    passed to the table-load pass is filtered (indices preserved) so
    both are served by the one table that holds exp AND ln -> a single
    ACT_TABLE_LOAD that overlaps the input DMAs.
"""

import numpy as np

N, D, Q, M = 2048, 256, 16, 64
NCORES = 8
NLOC = N // NCORES          # 256

# acqin column layout
C_QM = 0        # [0:32)    qmu  (chunk0 | chunk1)
C_QLS = 32      # [32:64)   q_log_sigma
C_AL = 64      # [64:96)   alpha tiled twice
C_CONST = 96    # [96:100)  [2*logvar, 4*logvar, 0, 0]
ACQ_W = 104

_compiled = None


def _patch_act_tables():
    """Filter the activation-table universe handed to the table-load
    pass so `natural_log_exp_and_others` is the only table providing
    Exp/Ln. Table indices (act_func_set_id) are preserved, so the
    emitted loads still point at the real act_info.json entry; the
    whole kernel then needs a single ACT_TABLE_LOAD."""
    import concourse.bacc as bacc_mod
    import concourse.mybir as mybir
    from concourse.hw_specs import get_activation_tables

    def patched(arch):
        real = get_activation_tables(arch)
        target = None
        for name, funcs in real.items():
            if (mybir.ActivationFunctionType.Exp in funcs
                    and mybir.ActivationFunctionType.Ln in funcs):
                target = name
                break
        if target is None:
            return real
        strip = {mybir.ActivationFunctionType.Exp,
                 mybir.ActivationFunctionType.Ln}
        return {
            name: (set(funcs) if name == target else set(funcs) - strip)
            for name, funcs in real.items()
        }

    bacc_mod.get_activation_tables = patched


def _build_bass():
    import concourse.bacc as bacc
    import concourse.mybir as mybir
    from concourse import masks
    from concourse.tile import TileContext

    _patch_act_tables()

    f32 = mybir.dt.float32
    f32r = mybir.dt.float32r
    AF = mybir.ActivationFunctionType
    OP = mybir.AluOpType
    AX = mybir.AxisListType

    nc = bacc.Bacc("TRN2", target_bir_lowering=False, num_swdge_queues=2)

    acqin_d = nc.declare_dram_parameter("acqin", [128, ACQ_W], f32, isOutput=False)
    zl_d = nc.declare_dram_parameter("zl", [34, 17 * 128], f32r, isOutput=False)
    y_d = nc.declare_dram_parameter("ybig", [128, 2 * NLOC], f32r, isOutput=False)
    zs1_d = nc.declare_dram_parameter("zs1x", [33, M], f32r, isOutput=False)
    a_o = nc.declare_dram_parameter("out_A", [M, D], f32, isOutput=True)
    st_o = nc.declare_dram_parameter("out_stats", [128, 25], f32, isOutput=True)

    with TileContext(nc) as tc:
        with (
            tc.tile_pool(name="const", bufs=1) as cpool,
            tc.tile_pool(name="big", bufs=1) as bigpool,
            tc.tile_pool(name="scr", bufs=2) as spool,
            tc.tile_pool(name="psum", bufs=3, space="PSUM") as ppool,
            tc.tile_pool(name="psums", bufs=1, space="PSUM") as ppools,
        ):
            # inputs. The 34-partition zl transfer (the per-partition-
            # bytes hog) is split: first 9 ij-chunks ride the sync (SP)
            # ring ahead of y, the rest go on the Scalar HW-DGE ring,
            # issued after the prep activations so the issue cost sits in
            # an ACT idle window.
            acqin = cpool.tile([128, ACQ_W], f32)
            nc.sync.dma_start(out=acqin[:, :], in_=acqin_d[:, :])
            zs1_sb = cpool.tile([33, M], f32r)
            nc.sync.dma_start(out=zs1_sb[:, :], in_=zs1_d[:, :])
            zl_sb = bigpool.tile([98, 17 * 128], f32r)
            zhalf = 10 * 128
            nc.sync.dma_start(out=zl_sb[64:98, :zhalf], in_=zl_d[:, :zhalf])
            nc.sync.dma_start(out=zl_sb[64:98, zhalf:], in_=zl_d[:, zhalf:])
            ybig = bigpool.tile([128, 2 * NLOC], f32r)
            nc.sync.dma_start(out=ybig[:, :], in_=y_d[:, :])

            ident = cpool.tile([128, 128], f32)
            masks.make_identity(nc, ident[:])

            qm = acqin[:, C_QM:C_QM + 32]
            qls = acqin[:, C_QLS:C_QLS + 32]
            al2 = acqin[:, C_AL:C_AL + 32]
            c2lv = acqin[:, C_CONST:C_CONST + 1]
            zs1x = zs1_sb[:, :]

            stats = bigpool.tile([128, 25], f32)
            nprep = bigpool.tile([98, 2 * 128], f32r)
            P = bigpool.tile([128, 196], f32)
            S = bigpool.tile([128, 96], f32)      # [d1(32) | d2(32) | qsig(32)]
            L = bigpool.tile([128, 96], f32)      # ln of S
            R = bigpool.tile([128, 64], f32)      # 1/d1, 1/d2
            M1 = bigpool.tile([128, 32], f32)
            scr2 = bigpool.tile([128, 64], f32)
            cols = bigpool.tile([128, 8], f32)

            qsig = S[:, 64:96]
            d1 = S[:, 0:32]
            d2 = S[:, 32:64]

            # q_sigma = softplus(qls) = ln(1 + exp(qls)), both chunks
            nc.scalar.activation(M1[:, :], qls, AF.Exp)
            nc.scalar.activation(qsig, M1[:, :], AF.Ln, bias=1.0)
            # d1 = alpha*qsig + 1 ; d2 = 2*d1 - 1
            nc.vector.tensor_mul(M1[:, :], qsig, al2)
            nc.vector.tensor_scalar_add(d1, M1[:, :], 1.0)
            nc.vector.tensor_scalar(out=d2, in0=d1, scalar1=2.0, scalar2=-1.0,
                                    op0=OP.mult, op1=OP.add)
            nc.vector.reciprocal(R[:, :], S[:, 0:64])
            # one Ln serves sum2 (ln d1), 2*s3 (ln d2) and the KL ln qsig
            nc.scalar.activation(L[:, :], S[:, :], AF.Ln)
            # SR cols: [sum2_c0, sum2_c1, s3x2_c0, s3x2_c1, lnsig_c0, lnsig_c1]
            nc.vector.tensor_reduce(
                stats[:, 17:23], L[:, :].rearrange("p (a b) -> p a b", b=16),
                axis=AX.X, op=OP.add)

            # both chunks at once via chunk-strided [p, 2, w] views of P
            # rows (post-transpose): 0:16 qmu*w1, 16:32 w1, 32 h1,
            # 33:64 zero, 64:80 qmu*w2, 80:96 w2, 96 one, 97 g
            Pv = P[:, :].rearrange("p (c s) -> p c s", c=2)
            qmv = qm.rearrange("p (c s) -> p c s", s=16)
            alv = al2.rearrange("p (c s) -> p c s", s=16)
            Rv1 = R[:, 0:32].rearrange("p (c s) -> p c s", s=16)
            Rv2 = R[:, 32:64].rearrange("p (c s) -> p c s", s=16)
            nc.vector.tensor_mul(Pv[:, :, 16:32], Rv1, alv)
            nc.vector.tensor_mul(Pv[:, :, 80:96], Rv2, alv)
            nc.vector.tensor_mul(Pv[:, :, 0:16], Pv[:, :, 16:32], qmv)
            nc.vector.tensor_mul(Pv[:, :, 64:80], Pv[:, :, 80:96], qmv)
            nc.vector.tensor_mul(
                scr2[:, 0:32].rearrange("p (c s) -> p c s", s=16),
                Pv[:, :, 0:16], qmv)
            nc.vector.tensor_mul(
                scr2[:, 32:64].rearrange("p (c s) -> p c s", s=16),
                Pv[:, :, 64:80], qmv)
            # cols: [rt1_c0, rt1_c1, a_c0, a_c1]
            nc.vector.tensor_reduce(
                cols[:, 0:4], scr2[:, :].rearrange("p (a b) -> p a b", b=16),
                axis=AX.X, op=OP.add)
            nc.vector.memset(Pv[:, :, 33:64], 0.0)
            nc.vector.memset(Pv[:, :, 96:97], 1.0)
            # h1 = 2*logvar - 0.5*(rt1 + sum2)
            nc.vector.tensor_add(cols[:, 4:6], cols[:, 0:2], stats[:, 17:19])
            nc.vector.tensor_scalar(
                out=Pv[:, :, 32:33],
                in0=cols[:, 4:6].rearrange("p (c s) -> p c s", s=1),
                scalar1=-0.5, scalar2=c2lv, op0=OP.mult, op1=OP.add)
            # g = -a - 0.5*(2*s3); 4*logvar rides in the zl "ones" row
            nc.vector.tensor_scalar(
                out=cols[:, 6:8], in0=stats[:, 19:21], scalar1=0.5,
                scalar2=0.0, op0=OP.mult, op1=OP.add)
            nc.vector.tensor_add(cols[:, 6:8], cols[:, 6:8], cols[:, 2:4])
            nc.vector.tensor_scalar(
                out=Pv[:, :, 97:98],
                in0=cols[:, 6:8].rearrange("p (c s) -> p c s", s=1),
                scalar1=-1.0, scalar2=0.0, op0=OP.mult, op1=OP.add)

            for c in range(2):
                ptp = ppools.tile([98, 128], f32, tag="ptp")
                nc.tensor.transpose(ptp[:, :], P[:, 98 * c:98 * (c + 1)],
                                    ident[:, :])
                nc.vector.tensor_copy(nprep[:, 128 * c:128 * (c + 1)], ptp[:, :])

            # psi1 exponent for both chunks into one PSUM tile, then exp
            # (borrows a rotation slot of the psi2 PSUM pool)
            e1 = ppool.tile([128, 4 * NLOC], f32, tag="p2")
            for c in range(2):
                nc.tensor.matmul(e1[:, M * c:M * (c + 1)],
                                 lhsT=nprep[0:33, 128 * c:128 * (c + 1)],
                                 rhs=zs1x,
                                 start=True, stop=True)
            psi1c = bigpool.tile([128, 2 * M], f32r)
            nc.scalar.activation(psi1c[:, :], e1[:, 0:2 * M], AF.Exp)

            # KL statistics (tr(y y^T) is an input-only reduction and is
            # done on the host)
            nc.vector.tensor_mul(scr2[:, 0:32], qm, qm)
            nc.vector.tensor_mul(scr2[:, 32:64], qsig, qsig)
            nc.vector.tensor_reduce(
                stats[:, 23:25], scr2[:, :].rearrange("p (a b) -> p a b", b=32),
                axis=AX.X, op=OP.add)

            # psi2: 17 ij-chunks in groups (first group small so the ACT
            # exp stream starts as early as possible); n-sums on DVE
            GROUPS = (2, 4, 4, 4, 2, 1)
            ch0 = 0
            for nch in GROUPS:
                w = nch * NLOC
                p2 = ppool.tile([128, 4 * NLOC], f32, tag="p2")
                for j in range(nch):
                    ch = ch0 + j
                    nc.tensor.matmul(
                        p2[:, j * NLOC:(j + 1) * NLOC],
                        lhsT=zl_sb[64:98, ch * 128:(ch + 1) * 128],
                        rhs=nprep[64:98, :],
                        start=True, stop=True)
                scr = spool.tile([128, 4 * NLOC], f32, tag="p2scr")
                nc.scalar.activation(scr[:, :w], p2[:, :w], AF.Exp)
                nc.vector.tensor_reduce(
                    stats[:, ch0:ch0 + nch],
                    scr[:, :w].rearrange("p (a b) -> p a b", b=NLOC),
                    axis=AX.X, op=OP.add)
                ch0 += nch

            # A = psi1^T y: PE work ordered after the psi2 matmuls so the
            # exp stream starts earlier; the DMA still overlaps the tail
            apsum = ppool.tile([128, 4 * NLOC], f32, tag="p2")
            for c in range(2):
                nc.tensor.matmul(apsum[0:M, 0:D],
                                 lhsT=psi1c[:, M * c:M * (c + 1)],
                                 rhs=ybig[:, NLOC * c:NLOC * (c + 1)],
                                 start=(c == 0), stop=(c == 1))
            a_sb = bigpool.tile([M, D], f32)
            nc.scalar.copy(a_sb[:, :], apsum[0:M, 0:D])
            nc.sync.dma_start(out=a_o[:, :], in_=a_sb[:, :])
            nc.sync.dma_start(out=st_o[:, :], in_=stats[:, :])

    nc.compile()
    return nc


def _get_compiled():
    global _compiled
    if _compiled is None:
        _compiled = _build_bass()
    return _compiled


def _np_softplus(x):
    return np.logaddexp(x, 0.0)


def kernel(y, q_mu, q_log_sigma, z, noise_raw, alpha, variance, _trace=False):
    from concourse.bass_utils import run_bass_kernel_spmd

    nc = _get_compiled()

    f8 = np.float64
    z64 = z.astype(f8)
    al = alpha.astype(f8)
    var = f8(variance[0])
    logvar = np.log(var)

    # z-side stationary blocks (host-built, replicated to all cores).
    # psi2 is symmetric in (i, j): ship only the 2080 upper-tri pairs.
    iu, ju = np.triu_indices(M)                             # (2080,)
    npairs = iu.shape[0]
    Su = z64[iu] + z64[ju]                                  # (2080, q)
    sqz = (z64[:, None, :] - z64[None, :, :]) ** 2          # (m, m, q)
    s1 = 0.25 * (sqz @ al)                                  # (m, m)
    zl = np.zeros((34, 17 * 128), np.float32)
    zl[0:16, :npairs] = Su.T
    zl[16:32, :npairs] = (-0.25 * Su * Su).T
    zl[32, :npairs] = -s1[iu, ju] + 4.0 * logvar
    zl[33, :npairs] = 1.0

    zt = z64.T                                              # (q, m)
    zs1x = np.zeros((33, M), np.float32)
    zs1x[0:16] = zt
    zs1x[16:32] = -0.5 * zt * zt
    zs1x[32] = 1.0

    qmu32 = q_mu.astype(np.float32)
    qls32 = q_log_sigma.astype(np.float32)
    y32 = y.astype(np.float32)

    in_maps = []
    for i in range(NCORES):
        r = i * NLOC
        acqin = np.zeros((128, ACQ_W), np.float32)
        acqin[:, C_QM:C_QM + 16] = qmu32[r:r + 128]
        acqin[:, C_QM + 16:C_QM + 32] = qmu32[r + 128:r + 256]
        acqin[:, C_QLS:C_QLS + 16] = qls32[r:r + 128]
        acqin[:, C_QLS + 16:C_QLS + 32] = qls32[r + 128:r + 256]
        acqin[:, C_AL:C_AL + 16] = alpha.reshape(1, Q)
        acqin[:, C_AL + 16:C_AL + 32] = alpha.reshape(1, Q)
        acqin[:, C_CONST] = 2.0 * logvar
        ybig = np.empty((128, 2 * NLOC), np.float32)
        ybig[:, 0:NLOC] = y32[r:r + 128]
        ybig[:, NLOC:2 * NLOC] = y32[r + 128:r + 256]
        in_maps.append({"acqin": acqin, "zl": zl, "ybig": ybig, "zs1x": zs1x})

    br = run_bass_kernel_spmd(nc, in_maps, list(range(NCORES)), trace=_trace)
    res = br.results

    stats = np.zeros((128, 25), f8)
    A = np.zeros((M, D), f8)
    for r in res:
        stats += r["out_stats"].astype(f8)
        A += r["out_A"].astype(f8)

    flat = stats[:, 0:17].T.reshape(17 * 128)
    psi2 = np.empty((M, M), f8)
    psi2[iu, ju] = flat[:npairs]
    psi2[ju, iu] = flat[:npairs]
    col = stats.sum(axis=0)
    lnsig = col[21] + col[22]
    musq = col[23]
    ssq = col[24]
    tr_yy = float(np.sum(y.astype(f8) ** 2))

    kl_sum = -lnsig + 0.5 * (ssq + musq) - 0.5 * N * Q
    kl_term = kl_sum / (N * D)

    # small m x m algebra on host
    k_mm = var * np.exp(-0.5 * (sqz @ al))                  # (m, m)
    noise_var = _np_softplus(f8(noise_raw[0]))
    beta = 1.0 / noise_var
    psi0 = N * var

    cov1 = beta * psi2 + k_mm
    B = np.linalg.solve(cov1, A)
    tr_yWy = beta * tr_yy - np.sum(A * B)

    F = 0.5 * N * np.log(beta)
    F += 0.5 * np.linalg.slogdet(k_mm)[1]
    F -= 0.5 * N * np.log(np.pi)
    F -= 0.5 * np.linalg.slogdet(cov1)[1]
    F -= 0.5 * beta * psi0
    F += 0.5 * np.trace(np.linalg.solve(k_mm, psi2))
    F = (F * D - 0.5 * tr_yWy) / (N * D)

    out = F - kl_term
    result = np.asarray(out, dtype=np.float32)
    if _trace:
        return result, br
    return result
